# revision 4
# baseline (speedup 1.0000x reference)
"""Bahdanau attention (B=32, S=2048, ENC2=1024, ATT=512) on 8 TRN2
NeuronCores, data-parallel over batch (4 batches/core), weights replicated.

v6: measured-rate rebalance. The PE on this silicon runs bf16 matmuls at
~2x the cost model (512-MM probe: 57.5us vs 109.2 modeled), so the PE
takes the enc transposes back from the DMA xbar (whose transposes cannot
overlap other DMA without a 2.3x mutual slowdown) and also does the
energy reduction directly:
  loads: plain f32 HWDGE DMA, free-running, 2 blocks ahead  (~38us)
  cast:  f32->bf16 split Pool (c0,c1) / DVE (c2,c3)
  encT:  per-[128,128] PE transposes (bf16), evac DVE/ACT
  UhT  = bf16(U)^T-chunk @ encT        [a 128, s 512] psum tiles
  tanh = tanh(UhT + Ws) via ACT bias fusion (per-partition bias)
  en   = sum_m v_m^T @ tanh_m          (4 direct PE matmuls per block:
         only two cross-engine hops ACT->PE->ACT on the energy path)
  alpha= exp(en)/sum exp(en)           (ACT exp + fused row-sum)

Output alpha [32, 2048] fp32, gathered from the 8 cores.
"""

import numpy as np

import concourse.bass as bass
import concourse.mybir as mybir
import concourse.tile as tile
from concourse import bacc
from concourse.masks import make_identity

F32 = mybir.dt.float32
F32R = mybir.dt.float32r
BF16 = mybir.dt.bfloat16

N_CORES = 8
B_FULL, S, E, A = 32, 2048, 1024, 512
B_SH = B_FULL // N_CORES          # 4 batches per core
SBLK = 512                        # s-block (matmul N)
N_SBLK = S // SBLK                # 4 per batch
EJ = E // 128                     # 8 e-chunks
AM = A // 128                     # 4 a-chunks
CC = SBLK // 128                  # 4 s-subchunks per s-block


def r(ap):
    return ap.bitcast(F32R)


def build_program(reps=1):
    nc = bacc.Bacc("TRN2", target_bir_lowering=False, debug=False,
                   num_devices=N_CORES)

    dec = nc.dram_tensor("decoder_hidden", [B_SH, E], F32R, kind="ExternalInput")
    enc = nc.dram_tensor("encoder_all_hidden", [B_SH, S, E], F32,
                         kind="ExternalInput")
    W_w = nc.dram_tensor("W_w", [E, A], F32R, kind="ExternalInput")
    W_b = nc.dram_tensor("W_b", [A], F32R, kind="ExternalInput")
    U_w = nc.dram_tensor("U_w", [E, A], F32R, kind="ExternalInput")
    U_b = nc.dram_tensor("U_b", [A], F32R, kind="ExternalInput")
    v_w = nc.dram_tensor("v_w", [A, 1], F32R, kind="ExternalInput")
    alpha = nc.dram_tensor("alpha", [B_SH, S], F32, kind="ExternalOutput")

    with tile.TileContext(nc) as tc:
        with (
            tc.tile_pool(name="const", bufs=1) as constp,
            tc.tile_pool(name="x4", bufs=3) as x4p,
            tc.tile_pool(name="x4b", bufs=2) as x4bp,
            tc.tile_pool(name="e8", bufs=3) as e8p,
            tc.tile_pool(name="tanh", bufs=9) as tanhp,
            tc.tile_pool(name="epi", bufs=2) as epip,
            tc.tile_pool(name="psT", bufs=3, space="PSUM") as psTp,
            tc.tile_pool(name="psUh", bufs=3, space="PSUM") as psUhp,
            tc.tile_pool(name="psE", bufs=2, space="PSUM") as psEp,
        ):
            # ---------------- prologue: tiny inputs ---------------------
            ident_f32 = constp.tile([128, 128], F32, tag="identf")
            make_identity(nc, ident_f32)
            ident = constp.tile([128, 128], F32R, tag="ident")
            nc.vector.tensor_copy(ident, ident_f32)
            identb = constp.tile([128, 128], BF16, tag="identb")
            nc.vector.tensor_copy(identb, ident_f32)

            dec_sb = constp.tile([B_SH, E], F32R, tag="dec")
            nc.sync.dma_start(dec_sb, dec[:, :])
            wb_sb = constp.tile([1, A], F32R, tag="wb")
            nc.sync.dma_start(wb_sb, W_b[None, :])
            ub_sb = constp.tile([1, A], F32R, tag="ub")
            nc.sync.dma_start(ub_sb, U_b[None, :])
            v_sb = constp.tile([128, AM], F32R, tag="v")
            nc.sync.dma_start(v_sb.rearrange("p (c o) -> p c o", c=AM),
                              v_w.rearrange("(c p) o -> p c o", p=128))

            # dec transposes first: PE work that's ready immediately
            dect = constp.tile([128, B_SH * EJ], F32R, tag="dect")
            for j in range(EJ):
                pst = psUhp.tile([128, SBLK], F32, tag="psUh",
                                 name=f"psTd_{j}").bitcast(F32R)
                nc.tensor.transpose(pst[:, :B_SH],
                                    r(dec_sb[:, 128 * j:128 * (j + 1)]),
                                    r(ident[:B_SH, :B_SH]))
                nc.scalar.copy(dect[:, B_SH * j:B_SH * (j + 1)], pst[:, :B_SH])

            # weights
            ww = constp.tile([128, EJ * A], F32R, tag="ww")
            nc.sync.dma_start(ww.rearrange("e (j a) -> e j a", j=EJ),
                              W_w.rearrange("(j e) a -> e j a", e=128))
            uw = constp.tile([128, EJ * A], F32R, tag="uw")
            uwb = constp.tile([128, EJ * A], BF16, tag="uwb")
            for j in range(EJ):
                sl = slice(A * j, A * (j + 1))
                nc.sync.dma_start(uw[:, sl], U_w[128 * j:128 * (j + 1), :])
                e0 = nc.gpsimd if j % 2 == 0 else nc.vector
                e0.tensor_copy(uwb[:, sl], uw.bitcast(F32)[:, sl])

            bias_sum = constp.tile([1, A], F32R, tag="bias")
            nc.vector.tensor_tensor(out=bias_sum, in0=wb_sb, in1=ub_sb,
                                    op=mybir.AluOpType.add)
            ones14f = constp.tile([1, B_SH], F32, tag="onesf")
            nc.vector.memset(ones14f, 1.0)
            ones14 = constp.tile([1, B_SH], F32R, tag="ones")
            nc.vector.tensor_copy(ones14, ones14f)

            wst = constp.tile([128, AM * B_SH], F32, tag="wst")

            def prologue_part2():
                # Ws = dec @ W_w + (W_b + U_b):  psum [B_SH, A]
                ps_ws = psEp.tile([B_SH, A], F32, tag="psE", name="ps_ws")
                for j in range(EJ):
                    nc.tensor.matmul(ps_ws,
                                     r(dect[:, B_SH * j:B_SH * (j + 1)]),
                                     r(ww[:, A * j:A * (j + 1)]),
                                     start=(j == 0), stop=False)
                nc.tensor.matmul(ps_ws, r(ones14), r(bias_sum),
                                 start=False, stop=True)
                ws_sb = constp.tile([B_SH, A], F32R, tag="ws", name="ws_sb")
                nc.scalar.copy(ws_sb, ps_ws)
                # WsT [128 a', (m b)]: col 4m+b = Ws[b, 128m + p]
                for m in range(AM):
                    pst = psUhp.tile([128, SBLK], F32, tag="psUh",
                                     name=f"pst_ws_{m}").bitcast(F32R)
                    nc.tensor.transpose(pst[:, :B_SH],
                                        r(ws_sb[:, 128 * m:128 * (m + 1)]),
                                        r(ident[:B_SH, :B_SH]))
                    nc.scalar.copy(wst[:, B_SH * m:B_SH * (m + 1)],
                                   pst[:, :B_SH])

            # ---------------- main loop ---------------------------------
            blocks = [(rep, b, sblk)
                      for rep in range(reps)
                      for b in range(B_SH)
                      for sblk in range(N_SBLK)]
            n = len(blocks)

            x4_tiles = {}
            x4b_tiles = {}
            e8_tiles = {}
            th_tiles = {}
            batch_state = {}   # (rep, b) -> (exp_b, den_b)
            pending = []       # block indices with tanh done, energy not

            def get_batch_state(rep, b):
                key = (rep, b)
                if key not in batch_state:
                    exp_b = epip.tile([1, S], F32, tag="exp",
                                      name=f"exp_{rep}_{b}")
                    den_b = epip.tile([1, N_SBLK], F32, tag="den",
                                      name=f"den_{rep}_{b}")
                    batch_state[key] = (exp_b, den_b)
                return batch_state[key]

            def issue_load(g):
                rep, b, sblk = blocks[g]
                s0 = SBLK * sblk
                x4 = x4p.tile([128, CC * E], F32, tag="x4",
                              name=f"x4_{g}")
                nc.sync.dma_start(
                    x4.rearrange("p (c e) -> p c e", c=CC),
                    enc[b, s0:s0 + SBLK, :]
                    .rearrange("(c p) e -> p c e", p=128))
                x4_tiles[g] = x4

            def issue_cast(g):
                # f32 -> bf16 split Pool / DVE
                x4 = x4_tiles.pop(g)
                x4b = x4bp.tile([128, CC * E], BF16, tag="x4b",
                                name=f"x4b_{g}")
                h = CC * E // 2
                nc.gpsimd.tensor_copy(x4b[:, :h], x4.bitcast(F32)[:, :h])
                nc.vector.tensor_copy(x4b[:, h:], x4.bitcast(F32)[:, h:])
                x4b_tiles[g] = x4b

            def transpose_evac(g, js):
                # PE transposes of [128,128] bf16 tiles; one psT tile per
                # j holds the 4 c-subchunks; evac to e8 (j s) layout
                if g not in e8_tiles:
                    e8 = e8p.tile([128, EJ * SBLK], BF16, tag="e8",
                                  name=f"e8_{g}")
                    e8_tiles[g] = e8
                else:
                    e8 = e8_tiles[g]
                x4b = x4b_tiles[g]
                for j in js:
                    pst = psTp.tile([128, SBLK], BF16, tag="psT")
                    for c in range(CC):
                        nc.tensor.transpose(
                            pst[:, 128 * c:128 * (c + 1)],
                            x4b[:, E * c + 128 * j:E * c + 128 * (j + 1)],
                            identb)
                    sl = slice(SBLK * j, SBLK * (j + 1))
                    if j in (2, 5):
                        nc.scalar.copy(e8[:, sl], pst)
                    else:
                        nc.vector.tensor_copy(e8[:, sl], pst)

            def flush_energy():
                g = pending.pop(0)
                rep, b, sblk = blocks[g]
                exp_b, den_b = get_batch_state(rep, b)
                s0 = SBLK * sblk
                ths = th_tiles.pop(g)
                # direct 4-matmul v-reduction on the (fast) PE: only two
                # cross-engine hops on the energy path
                ps_e = psEp.tile([1, SBLK], F32, tag="psE",
                                 name=f"psE_{rep}_{b}_{sblk}")
                for m in range(AM):
                    nc.tensor.matmul(ps_e, r(v_sb[:, m:m + 1]),
                                     r(ths[m]), start=(m == 0),
                                     stop=(m == AM - 1))
                nc.scalar.activation(
                    out=exp_b[:, s0:s0 + SBLK], in_=ps_e,
                    func=mybir.ActivationFunctionType.Exp,
                    accum_out=den_b[:, sblk:sblk + 1])
                if sblk == N_SBLK - 1:
                    finish_batch(rep, b, exp_b, den_b)

            def finish_batch(rep, b, exp_b, den_b):
                # softmax epilogue (no max subtraction; |energy| <= 22.6)
                dsum_b = epip.tile([1, 1], F32, tag="dsum",
                                   name=f"dsum_{rep}_{b}")
                nc.vector.reduce_sum(dsum_b, den_b,
                                     axis=mybir.AxisListType.X)
                inv_b = epip.tile([1, 1], F32, tag="inv",
                                  name=f"inv_{rep}_{b}")
                nc.vector.reciprocal(inv_b, dsum_b)
                h = S // 2
                nc.gpsimd.tensor_scalar_mul(exp_b[:, :h], exp_b[:, :h],
                                            inv_b)
                nc.vector.tensor_scalar_mul(exp_b[:, h:], exp_b[:, h:],
                                            inv_b)
                nc.sync.dma_start(alpha[b:b + 1, :h], exp_b[:, :h])
                nc.scalar.dma_start(alpha[b:b + 1, h:], exp_b[:, h:])
                del batch_state[(rep, b)]

            def mm_chain(g, m, ths):
                rep, b, sblk = blocks[g]
                e8 = e8_tiles[g]
                ps_uh = psUhp.tile([128, SBLK], F32, tag="psUh")
                for j in range(EJ):
                    nc.tensor.matmul(
                        ps_uh,
                        uwb[:, A * j + 128 * m:A * j + 128 * (m + 1)],
                        e8[:, SBLK * j:SBLK * (j + 1)],
                        start=(j == 0), stop=(j == EJ - 1))
                th = tanhp.tile([128, SBLK], F32R, tag="tanh",
                                name=f"tanh_{g}_{m}")
                nc.scalar.activation(
                    out=th, in_=ps_uh,
                    func=mybir.ActivationFunctionType.Tanh,
                    bias=wst[:, B_SH * m + b:B_SH * m + b + 1])
                ths.append(th)
                if m == AM - 1:
                    th_tiles[g] = ths
                    pending.append(g)
                    del e8_tiles[g]

            # pipeline: loads 2 ahead, cast 1 ahead; transposes of block
            # g+1 interleave between the MM chains of block g on the PE
            # queue so the cast has a chain-length window to land
            issue_load(0)
            issue_load(1)
            issue_cast(0)
            prologue_part2()
            transpose_evac(0, range(EJ))
            for g in range(n):
                if g + 2 < n:
                    issue_load(g + 2)
                if g + 1 < n:
                    issue_cast(g + 1)
                ths = []
                mm_chain(g, 0, ths)
                mm_chain(g, 1, ths)
                if g + 1 < n:
                    transpose_evac(g + 1, range(0, 4))
                mm_chain(g, 2, ths)
                mm_chain(g, 3, ths)
                if g + 1 < n:
                    transpose_evac(g + 1, range(4, EJ))
                if pending and (g >= 1 or len(pending) > 1):
                    flush_energy()
            while pending:
                flush_energy()

    nc.compile()
    return nc


def shard_inputs(inputs):
    """Full inputs dict -> list of 8 per-core input dicts."""
    dec = np.ascontiguousarray(inputs["decoder_hidden"], dtype=np.float32)
    enc = np.ascontiguousarray(inputs["encoder_all_hidden"], dtype=np.float32)
    base = {
        "W_w": np.ascontiguousarray(inputs["W_w"], dtype=np.float32),
        "W_b": np.ascontiguousarray(inputs["W_b"], dtype=np.float32),
        "U_w": np.ascontiguousarray(inputs["U_w"], dtype=np.float32),
        "U_b": np.ascontiguousarray(inputs["U_b"], dtype=np.float32),
        "v_w": np.ascontiguousarray(inputs["v_w"], dtype=np.float32),
    }
    maps = []
    for c in range(N_CORES):
        m = dict(base)
        m["decoder_hidden"] = dec[c * B_SH:(c + 1) * B_SH]
        m["encoder_all_hidden"] = enc[c * B_SH:(c + 1) * B_SH]
        maps.append(m)
    return maps


_NC_CACHE = None


def get_program():
    global _NC_CACHE
    if _NC_CACHE is None:
        _NC_CACHE = build_program()
    return _NC_CACHE


def kernel(**inputs):
    from concourse import bass_utils
    nc = get_program()
    maps = shard_inputs(inputs)
    res = bass_utils.run_bass_kernel_spmd(nc, maps,
                                          core_ids=list(range(N_CORES)))
    return np.concatenate([res.results[c]["alpha"] for c in range(N_CORES)],
                          axis=0)
